# revision 1
# baseline (speedup 1.0000x reference)
"""Deformable-conv-2d (adaptive dilation) Trainium2 Bass kernel.

Full-input contract: kernel(**inputs) takes the unsharded reference inputs
and returns the full (4, 256, 64, 64) float32 output.

Sharding: data-parallel over (batch sample x H-half) across 8 cores.
Core k handles sample b = k // 2, output rows [32*(k%2), 32*(k%2)+32).

Per-core device pipeline:
  1. small convs (offset 18ch / mask 9ch / adaptive-dilation 3ch) as 9
     shifted matmuls (fp32r) accumulated in PSUM.
  2. PE-transpose conv outputs to position-on-partition layout, then an
     elementwise fp32 pipeline (floor via mod, snap masks, clamps) produces
     per-sample-point gather indices + 4 bilinear corner coefficients.
  3. dma_gather (2KB rows = 2x2 corner patch x 256 ch, bf16) from an
     HBM table prepared on host.
  4. combine corners with scalar_tensor_tensor chains (coef = per-partition
     scalar), PE-transpose back to channel-major.
  5. big conv = bf16 matmuls (K = 256c x 9 taps) accumulating in PSUM.
"""

import numpy as np
import ml_dtypes

import concourse.bacc as bacc
import concourse.mybir as mybir
import concourse.tile as tile
from concourse.bass_utils import run_bass_kernel_spmd

DT = mybir.dt
ALU = mybir.AluOpType
ACTF = mybir.ActivationFunctionType

B, C, OC, H, W = 4, 256, 256, 64, 64
KS, PAD, DIL = 3, 1, 2
N = KS * KS                       # 9 kernel points
HP = H + 2 * PAD                  # 66 (reference x_pad height/width)
NCORES = 8
ROWS = H // 2                     # 32 output rows per core
S = ROWS * W                      # 2048 output positions per core
CHUNKS = S // 128                 # 16 s-chunks of 128
GROWS = HP * HP                   # 4356 gather-table rows
GE = 4 * C                        # 1024 bf16 elems per gather row

_CACHED = {}


def _build_program():
    nc = bacc.Bacc("TRN2", target_bir_lowering=False, debug=False,
                   num_devices=NCORES, num_swdge_queues=1)

    slab = nc.dram_tensor("slab", [2, 128, 34, HP], DT.float32r, kind="ExternalInput")
    rrows = nc.dram_tensor("rrows", [GROWS, GE], DT.bfloat16, kind="ExternalInput")
    wsm = nc.dram_tensor("wsm", [N, 2, 128, 30], DT.float32r, kind="ExternalInput")
    biasd = nc.dram_tensor("biasd", [30, 1], DT.float32, kind="ExternalInput")
    p0xd = nc.dram_tensor("p0xd", [128, CHUNKS, N], DT.float32, kind="ExternalInput")
    p0yd = nc.dram_tensor("p0yd", [128, CHUNKS, N], DT.float32, kind="ExternalInput")
    wcv = nc.dram_tensor("wcv", [N, 2, 128, OC], DT.bfloat16, kind="ExternalInput")
    eyebd = nc.dram_tensor("eyebd", [128, 128], DT.bfloat16, kind="ExternalInput")
    eyefd = nc.dram_tensor("eyefd", [128, 128], DT.float32, kind="ExternalInput")
    outd = nc.dram_tensor("out", [2, 128, S], DT.float32, kind="ExternalOutput")

    f32r = DT.float32r

    with tile.TileContext(nc) as tc:
        with (
            tc.tile_pool(name="const", bufs=1) as cpool,
            tc.tile_pool(name="work", bufs=1) as wpool,
            tc.tile_pool(name="pipe", bufs=1) as ppool,
            tc.tile_pool(name="gath", bufs=3) as gpool,
            tc.tile_pool(name="xo", bufs=2) as xopool,
            tc.tile_pool(name="xot", bufs=4) as xotpool,
            tc.tile_pool(name="osb", bufs=2) as opool,
            tc.tile_pool(name="acc", bufs=1, space="PSUM") as acc_pool,
            tc.tile_pool(name="tp", bufs=2, space="PSUM") as tp_pool,
        ):
            # ---- static loads -------------------------------------------------
            slab_sb = []
            for g in range(2):
                sl = cpool.tile([128, 34, HP], DT.float32r, tag=f"slab{g}",
                                name=f"slab_sb{g}")
                nc.sync.dma_start(sl[:], slab[g])
                slab_sb.append(sl)

            wsm_sb = cpool.tile([128, N, 2, 30], DT.float32r, tag="wsm")
            nc.sync.dma_start(wsm_sb[:], wsm.ap().transpose([2, 0, 1, 3]))
            wcv_sb = cpool.tile([128, N, 2, OC], DT.bfloat16, tag="wcv")
            nc.sync.dma_start(wcv_sb[:], wcv.ap().transpose([2, 0, 1, 3]))

            bias_sb = cpool.tile([30, 1], DT.float32, tag="bias")
            nc.sync.dma_start(bias_sb[:], biasd.ap())
            p0x_sb = cpool.tile([128, CHUNKS, N], DT.float32, tag="p0x")
            nc.sync.dma_start(p0x_sb[:], p0xd.ap())
            p0y_sb = cpool.tile([128, CHUNKS, N], DT.float32, tag="p0y")
            nc.sync.dma_start(p0y_sb[:], p0yd.ap())
            eyeb = cpool.tile([128, 128], DT.bfloat16, tag="eyeb")
            nc.sync.dma_start(eyeb[:], eyebd.ap())
            eyef = cpool.tile([128, 128], DT.float32, tag="eyef")
            nc.sync.dma_start(eyef[:], eyefd.ap())

            # ---- small convs: PSUM [30, 2048], 9 taps x 2 cgroups ------------
            psc = acc_pool.tile([30, 4 * 512], DT.float32, tag="acc", name="psc")
            for blk in range(4):
                first = True
                for t in range(N):
                    ki, kj = t // 3, t % 3
                    for g in range(2):
                        rhs = slab_sb[g][:, blk * 8 + ki: blk * 8 + ki + 8,
                                         kj: kj + W]
                        nc.tensor.matmul(
                            psc[:, blk * 512:(blk + 1) * 512],
                            wsm_sb[:, t, g, :],
                            rhs,
                            start=first, stop=(t == N - 1 and g == 1))
                        first = False

            conv_sb = wpool.tile([30, S], DT.float32, tag="conv")
            nc.scalar.activation(conv_sb[:], psc[:], ACTF.Identity, bias=bias_sb[:])

            # transpose -> convT [128, CHUNKS, 30]
            pst = tp_pool.tile([128, 480], DT.float32, tag="tp", name="pst")
            for ch in range(CHUNKS):
                nc.tensor.transpose(pst[:, ch * 30:(ch + 1) * 30],
                                    conv_sb[:, ch * 128:(ch + 1) * 128],
                                    eyef[:30, :30])
            convT = wpool.tile([128, CHUNKS, 30], DT.float32, tag="convT")
            nc.vector.tensor_copy(convT.rearrange("p a b -> p (a b)"), pst[:])

            # ---- index & coefficient pipeline (fp32) -------------------------
            def t9(tag):
                return ppool.tile([128, CHUNKS, N], DT.float32, tag=tag, name=tag)

            def t3_(tag):
                return ppool.tile([128, CHUNKS, 3], DT.float32, tag=tag, name=tag)

            m9 = t9("m9")
            nc.scalar.activation(m9[:], convT[:, :, 18:27], ACTF.Sigmoid)
            adb = t3_("adb")
            nc.scalar.activation(adb[:], convT[:, :, 27:30], ACTF.Sigmoid,
                                 scale=-1.0)
            t3 = t3_("t3")
            nc.vector.tensor_scalar(t3[:], adb[:], 2.0, 1.0, ALU.mult, ALU.add)
            adm = t3_("adm")
            nc.vector.tensor_scalar(adm[:], t3[:], 2.0, -4.0, ALU.mult, ALU.add)
            mu = t9("mu")
            for k in range(3):
                nc.vector.tensor_tensor(mu[:, :, 3 * k:3 * k + 3],
                                        m9[:, :, 3 * k:3 * k + 3],
                                        adm[:, :, 0:3], ALU.mult)

            def axis_pipeline(off_lo, p0_sb, is_x, tagp):
                p = t9(tagp + "p")
                nc.vector.tensor_tensor(p[:], convT[:, :, off_lo:off_lo + N],
                                        p0_sb[:], ALU.add)
                if is_x:
                    nc.vector.tensor_tensor(p[:, :, 0:3], p[:, :, 0:3],
                                            t3[:, :, 0:3], ALU.subtract)
                    nc.vector.tensor_tensor(p[:, :, 6:9], p[:, :, 6:9],
                                            t3[:, :, 0:3], ALU.add)
                else:
                    for n in (0, 3, 6):
                        nc.vector.tensor_tensor(p[:, :, n:n + 1], p[:, :, n:n + 1],
                                                t3[:, :, 0:1], ALU.subtract)
                    for n in (2, 5, 8):
                        nc.vector.tensor_tensor(p[:, :, n:n + 1], p[:, :, n:n + 1],
                                                t3[:, :, 2:3], ALU.add)
                nc.vector.tensor_scalar(p[:], p[:], -10.0, 76.0, ALU.max, ALU.min)
                tmp = t9(tagp + "tmp")
                # floor(p): t = int-round(p) (any within-1 rounding), then
                # f = t - (t > p)
                ti = ppool.tile([128, CHUNKS, N], DT.int32, tag=tagp + "ti",
                                name=tagp + "ti")
                nc.vector.tensor_copy(ti[:], p[:])
                f = t9(tagp + "f")
                nc.vector.tensor_copy(f[:], ti[:])
                nc.vector.tensor_tensor(tmp[:], f[:], p[:], ALU.is_gt)
                nc.vector.tensor_tensor(f[:], f[:], tmp[:], ALU.subtract)
                ma = t9(tagp + "ma")
                nc.vector.tensor_single_scalar(ma[:], p[:], 1.0, ALU.is_lt)
                nc.vector.tensor_single_scalar(tmp[:], p[:], float(HP - 2),
                                               ALU.is_gt)
                nc.vector.tensor_tensor(ma[:], ma[:], tmp[:], ALU.add)
                # snapped & clipped p_used
                nc.vector.tensor_tensor(tmp[:], f[:], p[:], ALU.subtract)
                nc.vector.tensor_tensor(tmp[:], ma[:], tmp[:], ALU.mult)
                pu = t9(tagp + "pu")
                nc.vector.tensor_tensor(pu[:], p[:], tmp[:], ALU.add)
                nc.vector.tensor_scalar(pu[:], pu[:], 0.0, float(HP - 1),
                                        ALU.max, ALU.min)
                ql = t9(tagp + "ql")
                nc.vector.tensor_scalar(ql[:], f[:], 0.0, float(HP - 1),
                                        ALU.max, ALU.min)
                qr = t9(tagp + "qr")
                nc.vector.tensor_scalar(qr[:], f[:], 1.0, 0.0, ALU.add, ALU.max)
                nc.vector.tensor_single_scalar(qr[:], qr[:], float(HP - 1),
                                               ALU.min)
                ex = t9(tagp + "ex")
                nc.vector.tensor_tensor(ex[:], qr[:], ql[:], ALU.is_equal)
                wl = t9(tagp + "wl")
                nc.vector.tensor_tensor(wl[:], ql[:], pu[:], ALU.subtract)
                nc.vector.tensor_single_scalar(wl[:], wl[:], 1.0, ALU.add)
                wrt = t9(tagp + "wrt")
                nc.vector.tensor_tensor(wrt[:], pu[:], qr[:], ALU.subtract)
                nc.vector.tensor_single_scalar(wrt[:], wrt[:], 1.0, ALU.add)
                nc.vector.tensor_tensor(tmp[:], wrt[:], ex[:], ALU.mult)
                c0 = t9(tagp + "c0")
                nc.vector.tensor_tensor(c0[:], wl[:], tmp[:], ALU.add)
                c1 = t9(tagp + "c1")
                nc.vector.tensor_tensor(c1[:], wrt[:], tmp[:], ALU.subtract)
                return ql, c0, c1

            qlx, cx0, cx1 = axis_pipeline(0, p0x_sb, True, "x")
            qly, cy0, cy1 = axis_pipeline(N, p0y_sb, False, "y")

            w0 = t9("w0")
            nc.vector.tensor_tensor(w0[:], mu[:], cx0[:], ALU.mult)
            w1 = t9("w1")
            nc.vector.tensor_tensor(w1[:], mu[:], cx1[:], ALU.mult)
            u00 = t9("u00")
            nc.vector.tensor_tensor(u00[:], w0[:], cy0[:], ALU.mult)
            u01 = t9("u01")
            nc.vector.tensor_tensor(u01[:], w0[:], cy1[:], ALU.mult)
            u10 = t9("u10")
            nc.vector.tensor_tensor(u10[:], w1[:], cy0[:], ALU.mult)
            u11 = t9("u11")
            nc.vector.tensor_tensor(u11[:], w1[:], cy1[:], ALU.mult)

            idxf = t9("idxf")
            nc.vector.scalar_tensor_tensor(idxf[:], qlx[:], float(HP), qly[:],
                                           ALU.mult, ALU.add)
            idx16 = ppool.tile([128, CHUNKS, N], DT.int16, tag="idx16")
            nc.vector.tensor_copy(idx16[:], idxf[:])
            idx16b = ppool.tile([128, N, CHUNKS], DT.int16, tag="idx16b")
            nc.vector.tensor_copy(idx16b[:], idx16.transpose([0, 2, 1]))

            # ---- wrapped-index relayout (idx i at partition i%16, free i//16)
            wr = wpool.tile([128, N * 128], DT.int16, tag="wr")
            wr4 = wr.rearrange("p (n c e) -> p n c e", n=N, c=CHUNKS, e=8)
            for pg in range(8):
                nc.sync.dma_start(
                    wr4[0:16, :, :, pg],
                    idx16b[16 * pg:16 * pg + 16])
            for g in range(1, 8):
                nc.sync.dma_start(wr[16 * g:16 * g + 16, :], wr[0:16, :])

            # ---- main loop ----------------------------------------------------
            pso = acc_pool.tile([128, 2 * 1024], DT.float32, tag="acc", name="pso")
            for sh in range(2):
                for n in range(N):
                    gb = gpool.tile([128, 8, GE], DT.bfloat16, tag="gb", name="gb")
                    nc.gpsimd.dma_gather(
                        gb[:], rrows.ap(),
                        wr[:, n * 128 + sh * 64: n * 128 + sh * 64 + 64],
                        num_idxs=1024, num_idxs_reg=1024, elem_size=GE,
                        queue_num=0)
                    # corner combine: ACT does the first scale, DVE fused
                    # scalar_tensor_tensor (mul+add) chains the rest
                    xos = []
                    for ch in range(8):
                        cidx = sh * 8 + ch
                        xo = xopool.tile([128, C], DT.bfloat16, tag="xo",
                                         name="xo", bufs=10)
                        nc.scalar.mul(xo[:], gb[:, ch, 0:C],
                                      u00[:, cidx, n:n + 1])
                        nc.vector.scalar_tensor_tensor(
                            xo[:], gb[:, ch, C:2 * C], u01[:, cidx, n:n + 1],
                            xo[:], ALU.mult, ALU.add)
                        nc.vector.scalar_tensor_tensor(
                            xo[:], gb[:, ch, 2 * C:3 * C], u10[:, cidx, n:n + 1],
                            xo[:], ALU.mult, ALU.add)
                        nc.vector.scalar_tensor_tensor(
                            xo[:], gb[:, ch, 3 * C:4 * C], u11[:, cidx, n:n + 1],
                            xo[:], ALU.mult, ALU.add)
                        xos.append(xo)
                    for g in range(2):
                        pt = tp_pool.tile([128, 1024], DT.bfloat16, tag="tp",
                                          name="pt")
                        for ch in range(8):
                            nc.tensor.transpose(pt[:, ch * 128:(ch + 1) * 128],
                                                xos[ch][:, g * 128:(g + 1) * 128],
                                                eyeb[:])
                        xoT = xotpool.tile([128, 1024], DT.bfloat16, tag="xoT",
                                           name="xoT")
                        nc.scalar.copy(xoT[:], pt[:])
                        for og in range(2):
                            for nb in range(2):
                                nc.tensor.matmul(
                                    pso[:, og * 1024 + nb * 512:
                                        og * 1024 + (nb + 1) * 512],
                                    wcv_sb[:, n, g, og * 128:(og + 1) * 128],
                                    xoT[:, nb * 512:(nb + 1) * 512],
                                    start=(n == 0 and g == 0),
                                    stop=(n == N - 1 and g == 1))
                for og in range(2):
                    ob = opool.tile([128, 1024], DT.float32, tag="ob", name="ob")
                    nc.scalar.copy(ob[:], pso[:, og * 1024:(og + 1) * 1024])
                    nc.sync.dma_start(outd[og, :, sh * 1024:(sh + 1) * 1024], ob[:])

    nc.compile()
    return nc


def _host_prep(x, w_conv, w_p, b_p, w_m, b_m, w_ad, b_ad):
    bf16 = ml_dtypes.bfloat16
    x = np.asarray(x, dtype=np.float32)
    wsm_full = np.concatenate([np.asarray(w_p), np.asarray(w_m),
                               np.asarray(w_ad)], axis=0).astype(np.float32)
    wsm_in = np.ascontiguousarray(
        wsm_full.transpose(2, 3, 1, 0).reshape(N, 2, 128, 30))
    bias_in = np.concatenate([np.asarray(b_p), np.asarray(b_m),
                              np.asarray(b_ad)]).astype(np.float32).reshape(30, 1)
    wcv_in = np.ascontiguousarray(
        np.asarray(w_conv).astype(np.float32).transpose(2, 3, 1, 0)
        .reshape(N, 2, 128, OC)).astype(bf16)
    eyeb = np.eye(128, dtype=np.float32).astype(bf16)
    eyef = np.eye(128, dtype=np.float32)

    in_maps = []
    for k in range(NCORES):
        b, half = k // 2, k % 2
        i0 = ROWS * half
        xp = np.pad(x[b], ((0, 0), (1, 1), (1, 1)))
        slab = np.ascontiguousarray(xp[:, i0:i0 + 34, :]).reshape(2, 128, 34, HP)
        a = np.pad(x[b], ((0, 0), (1, 2), (1, 2))).astype(bf16)
        t = a.transpose(1, 2, 0)                       # (67, 67, 256)
        r4 = np.empty((HP, HP, 4, C), dtype=bf16)
        r4[:, :, 0] = t[0:HP, 0:HP]
        r4[:, :, 1] = t[0:HP, 1:HP + 1]
        r4[:, :, 2] = t[1:HP + 1, 0:HP]
        r4[:, :, 3] = t[1:HP + 1, 1:HP + 1]
        rr = r4.reshape(GROWS, GE)
        sidx = np.arange(S)
        p0x = (1.0 + i0 + sidx // W).astype(np.float32)
        p0y = (1.0 + sidx % W).astype(np.float32)
        # layout [partition, chunk, n]: s = chunk*128 + p
        p0x_t = np.ascontiguousarray(
            np.broadcast_to(p0x.reshape(CHUNKS, 128).T[:, :, None],
                            (128, CHUNKS, N)))
        p0y_t = np.ascontiguousarray(
            np.broadcast_to(p0y.reshape(CHUNKS, 128).T[:, :, None],
                            (128, CHUNKS, N)))
        in_maps.append({
            "slab": slab.astype(np.float32),
            "rrows": rr,
            "wsm": wsm_in,
            "biasd": bias_in,
            "p0xd": p0x_t,
            "p0yd": p0y_t,
            "wcv": wcv_in,
            "eyebd": eyeb,
            "eyefd": eyef,
        })
    return in_maps


def _assemble(results):
    out = np.empty((B, OC, H, W), dtype=np.float32)
    for k in range(NCORES):
        b, half = k // 2, k % 2
        i0 = ROWS * half
        o = np.asarray(results[k]["out"], dtype=np.float32)   # (2, 128, S)
        out[b, :, i0:i0 + ROWS, :] = o.reshape(OC, ROWS, W)
    return out


def run_kernel(inputs, trace=False, **trace_kwargs):
    if "nc" not in _CACHED:
        _CACHED["nc"] = _build_program()
    nc = _CACHED["nc"]
    in_maps = _host_prep(**inputs)
    res = run_bass_kernel_spmd(nc, in_maps, list(range(NCORES)), trace=trace,
                               **trace_kwargs)
    return _assemble(res.results), res


def kernel(**inputs) -> np.ndarray:
    out, _ = run_kernel(inputs)
    return out



# revision 4
# speedup vs baseline: 1.0006x; 1.0006x over previous
"""Deformable-conv-2d (adaptive dilation) Trainium2 Bass kernel.

Full-input contract: kernel(**inputs) takes the unsharded reference inputs
and returns the full (4, 256, 64, 64) float32 output.

Sharding: data-parallel over (batch sample x H-half) across 8 cores.
Core k handles sample b = k // 2, output rows [32*(k%2), 32*(k%2)+32).

Per-core device pipeline:
  1. small convs (offset 18ch / mask 9ch / adaptive-dilation 3ch) as 9
     shifted matmuls (fp32r) accumulated in PSUM.
  2. PE-transpose conv outputs to position-on-partition layout, then an
     elementwise fp32 pipeline (floor via mod, snap masks, clamps) produces
     per-sample-point gather indices + 4 bilinear corner coefficients.
  3. dma_gather (2KB rows = 2x2 corner patch x 256 ch, bf16) from an
     HBM table prepared on host.
  4. combine corners with scalar_tensor_tensor chains (coef = per-partition
     scalar), PE-transpose back to channel-major.
  5. big conv = bf16 matmuls (K = 256c x 9 taps) accumulating in PSUM.
"""

import numpy as np
import ml_dtypes

import concourse.bacc as bacc
import concourse.mybir as mybir
import concourse.tile as tile
from concourse.bass_utils import run_bass_kernel_spmd

DT = mybir.dt
ALU = mybir.AluOpType
ACTF = mybir.ActivationFunctionType

B, C, OC, H, W = 4, 256, 256, 64, 64
KS, PAD, DIL = 3, 1, 2
N = KS * KS                       # 9 kernel points
HP = H + 2 * PAD                  # 66 (reference x_pad height/width)
NCORES = 8
ROWS = H // 2                     # 32 output rows per core
S = ROWS * W                      # 2048 output positions per core
CHUNKS = S // 128                 # 16 s-chunks of 128
GROWS = HP * HP                   # 4356 gather-table rows
GE = 4 * C                        # 1024 bf16 elems per gather row

_CACHED = {}


def _build_program():
    nc = bacc.Bacc("TRN2", target_bir_lowering=False, debug=False,
                   num_devices=NCORES, num_swdge_queues=1)

    slab = nc.dram_tensor("slab", [2, 128, 34, HP], DT.float32r, kind="ExternalInput")
    rrows = nc.dram_tensor("rrows", [GROWS, GE], DT.bfloat16, kind="ExternalInput")
    wsm = nc.dram_tensor("wsm", [N, 2, 128, 30], DT.float32r, kind="ExternalInput")
    biasd = nc.dram_tensor("biasd", [30, 1], DT.float32, kind="ExternalInput")
    p0xd = nc.dram_tensor("p0xd", [128, CHUNKS, N], DT.float32, kind="ExternalInput")
    p0yd = nc.dram_tensor("p0yd", [128, CHUNKS, N], DT.float32, kind="ExternalInput")
    wcv = nc.dram_tensor("wcv", [N, 2, 128, OC], DT.bfloat16, kind="ExternalInput")
    eyebd = nc.dram_tensor("eyebd", [128, 128], DT.bfloat16, kind="ExternalInput")
    eyefd = nc.dram_tensor("eyefd", [128, 128], DT.float32, kind="ExternalInput")
    outd = nc.dram_tensor("out", [2, 128, S], DT.float32, kind="ExternalOutput")

    f32r = DT.float32r

    with tile.TileContext(nc) as tc:
        with (
            tc.tile_pool(name="const", bufs=1) as cpool,
            tc.tile_pool(name="work", bufs=1) as wpool,
            tc.tile_pool(name="pipe", bufs=1) as ppool,
            tc.tile_pool(name="gath", bufs=4) as gpool,
            tc.tile_pool(name="xo", bufs=2) as xopool,
            tc.tile_pool(name="xot", bufs=4) as xotpool,
            tc.tile_pool(name="osb", bufs=2) as opool,
            tc.tile_pool(name="acc", bufs=1, space="PSUM") as acc_pool,
            tc.tile_pool(name="tp", bufs=2, space="PSUM") as tp_pool,
        ):
            # ---- static loads -------------------------------------------------
            slab_sb = []
            for g in range(2):
                sl = cpool.tile([128, 34, HP], DT.float32r, tag=f"slab{g}",
                                name=f"slab_sb{g}")
                nc.sync.dma_start(sl[:], slab[g])
                slab_sb.append(sl)

            wsm_sb = cpool.tile([128, N, 2, 30], DT.float32r, tag="wsm")
            nc.sync.dma_start(wsm_sb[:], wsm.ap().transpose([2, 0, 1, 3]))
            wcv_sb = cpool.tile([128, N, 2, OC], DT.bfloat16, tag="wcv")
            nc.sync.dma_start(wcv_sb[:], wcv.ap().transpose([2, 0, 1, 3]))

            bias_sb = cpool.tile([30, 1], DT.float32, tag="bias")
            nc.sync.dma_start(bias_sb[:], biasd.ap())
            p0x_sb = cpool.tile([128, CHUNKS, N], DT.float32, tag="p0x")
            nc.sync.dma_start(p0x_sb[:], p0xd.ap())
            p0y_sb = cpool.tile([128, CHUNKS, N], DT.float32, tag="p0y")
            nc.sync.dma_start(p0y_sb[:], p0yd.ap())
            eyeb = cpool.tile([128, 128], DT.bfloat16, tag="eyeb")
            nc.sync.dma_start(eyeb[:], eyebd.ap())
            eyef = cpool.tile([128, 128], DT.float32, tag="eyef")
            nc.sync.dma_start(eyef[:], eyefd.ap())

            # ---- small convs: PSUM [30, 2048], 9 taps x 2 cgroups ------------
            psc = acc_pool.tile([30, 4 * 512], DT.float32, tag="acc", name="psc")
            for blk in range(4):
                first = True
                for t in range(N):
                    ki, kj = t // 3, t % 3
                    for g in range(2):
                        rhs = slab_sb[g][:, blk * 8 + ki: blk * 8 + ki + 8,
                                         kj: kj + W]
                        nc.tensor.matmul(
                            psc[:, blk * 512:(blk + 1) * 512],
                            wsm_sb[:, t, g, :],
                            rhs,
                            start=first, stop=(t == N - 1 and g == 1))
                        first = False

            conv_sb = wpool.tile([30, S], DT.float32, tag="conv")
            nc.scalar.activation(conv_sb[:], psc[:], ACTF.Identity, bias=bias_sb[:])

            # transpose -> convT [128, CHUNKS, 30]
            pst = tp_pool.tile([128, 480], DT.float32, tag="tp", name="pst")
            for ch in range(CHUNKS):
                nc.tensor.transpose(pst[:, ch * 30:(ch + 1) * 30],
                                    conv_sb[:, ch * 128:(ch + 1) * 128],
                                    eyef[:30, :30])
            convT = wpool.tile([128, CHUNKS, 30], DT.float32, tag="convT")
            nc.vector.tensor_copy(convT.rearrange("p a b -> p (a b)"), pst[:])

            # ---- index & coefficient pipeline (fp32) -------------------------
            def t9(tag):
                return ppool.tile([128, CHUNKS, N], DT.float32, tag=tag, name=tag)

            def t3_(tag):
                return ppool.tile([128, CHUNKS, 3], DT.float32, tag=tag, name=tag)

            m9 = t9("m9")
            nc.scalar.activation(m9[:], convT[:, :, 18:27], ACTF.Sigmoid)
            adb = t3_("adb")
            nc.scalar.activation(adb[:], convT[:, :, 27:30], ACTF.Sigmoid,
                                 scale=-1.0)
            t3 = t3_("t3")
            nc.vector.tensor_scalar(t3[:], adb[:], 2.0, 1.0, ALU.mult, ALU.add)
            adm = t3_("adm")
            nc.vector.tensor_scalar(adm[:], t3[:], 2.0, -4.0, ALU.mult, ALU.add)
            mu = t9("mu")
            for k in range(3):
                nc.vector.tensor_tensor(mu[:, :, 3 * k:3 * k + 3],
                                        m9[:, :, 3 * k:3 * k + 3],
                                        adm[:, :, 0:3], ALU.mult)

            def axis_pipeline(off_lo, p0_sb, is_x, tagp):
                p = t9(tagp + "p")
                nc.vector.tensor_tensor(p[:], convT[:, :, off_lo:off_lo + N],
                                        p0_sb[:], ALU.add)
                if is_x:
                    nc.vector.tensor_tensor(p[:, :, 0:3], p[:, :, 0:3],
                                            t3[:, :, 0:3], ALU.subtract)
                    nc.vector.tensor_tensor(p[:, :, 6:9], p[:, :, 6:9],
                                            t3[:, :, 0:3], ALU.add)
                else:
                    for n in (0, 3, 6):
                        nc.vector.tensor_tensor(p[:, :, n:n + 1], p[:, :, n:n + 1],
                                                t3[:, :, 0:1], ALU.subtract)
                    for n in (2, 5, 8):
                        nc.vector.tensor_tensor(p[:, :, n:n + 1], p[:, :, n:n + 1],
                                                t3[:, :, 2:3], ALU.add)
                nc.vector.tensor_scalar(p[:], p[:], -10.0, 76.0, ALU.max, ALU.min)
                tmp = t9(tagp + "tmp")
                # floor(p): t = int-round(p) (any within-1 rounding), then
                # f = t - (t > p)
                ti = ppool.tile([128, CHUNKS, N], DT.int32, tag=tagp + "ti",
                                name=tagp + "ti")
                nc.vector.tensor_copy(ti[:], p[:])
                f = t9(tagp + "f")
                nc.vector.tensor_copy(f[:], ti[:])
                nc.vector.tensor_tensor(tmp[:], f[:], p[:], ALU.is_gt)
                nc.vector.tensor_tensor(f[:], f[:], tmp[:], ALU.subtract)
                ma = t9(tagp + "ma")
                nc.vector.tensor_single_scalar(ma[:], p[:], 1.0, ALU.is_lt)
                nc.vector.tensor_single_scalar(tmp[:], p[:], float(HP - 2),
                                               ALU.is_gt)
                nc.vector.tensor_tensor(ma[:], ma[:], tmp[:], ALU.add)
                # snapped & clipped p_used
                nc.vector.tensor_tensor(tmp[:], f[:], p[:], ALU.subtract)
                nc.vector.tensor_tensor(tmp[:], ma[:], tmp[:], ALU.mult)
                pu = t9(tagp + "pu")
                nc.vector.tensor_tensor(pu[:], p[:], tmp[:], ALU.add)
                nc.vector.tensor_scalar(pu[:], pu[:], 0.0, float(HP - 1),
                                        ALU.max, ALU.min)
                ql = t9(tagp + "ql")
                nc.vector.tensor_scalar(ql[:], f[:], 0.0, float(HP - 1),
                                        ALU.max, ALU.min)
                qr = t9(tagp + "qr")
                nc.vector.tensor_scalar(qr[:], f[:], 1.0, 0.0, ALU.add, ALU.max)
                nc.vector.tensor_single_scalar(qr[:], qr[:], float(HP - 1),
                                               ALU.min)
                ex = t9(tagp + "ex")
                nc.vector.tensor_tensor(ex[:], qr[:], ql[:], ALU.is_equal)
                wl = t9(tagp + "wl")
                nc.vector.tensor_tensor(wl[:], ql[:], pu[:], ALU.subtract)
                nc.vector.tensor_single_scalar(wl[:], wl[:], 1.0, ALU.add)
                wrt = t9(tagp + "wrt")
                nc.vector.tensor_tensor(wrt[:], pu[:], qr[:], ALU.subtract)
                nc.vector.tensor_single_scalar(wrt[:], wrt[:], 1.0, ALU.add)
                nc.vector.tensor_tensor(tmp[:], wrt[:], ex[:], ALU.mult)
                c0 = t9(tagp + "c0")
                nc.vector.tensor_tensor(c0[:], wl[:], tmp[:], ALU.add)
                c1 = t9(tagp + "c1")
                nc.vector.tensor_tensor(c1[:], wrt[:], tmp[:], ALU.subtract)
                return ql, c0, c1

            qlx, cx0, cx1 = axis_pipeline(0, p0x_sb, True, "x")
            qly, cy0, cy1 = axis_pipeline(N, p0y_sb, False, "y")

            w0 = t9("w0")
            nc.vector.tensor_tensor(w0[:], mu[:], cx0[:], ALU.mult)
            w1 = t9("w1")
            nc.vector.tensor_tensor(w1[:], mu[:], cx1[:], ALU.mult)
            u00 = t9("u00")
            nc.vector.tensor_tensor(u00[:], w0[:], cy0[:], ALU.mult)
            u01 = t9("u01")
            nc.vector.tensor_tensor(u01[:], w0[:], cy1[:], ALU.mult)
            u10 = t9("u10")
            nc.vector.tensor_tensor(u10[:], w1[:], cy0[:], ALU.mult)
            u11 = t9("u11")
            nc.vector.tensor_tensor(u11[:], w1[:], cy1[:], ALU.mult)

            def tb(tag, src):
                t = ppool.tile([128, CHUNKS, N], DT.bfloat16, tag=tag, name=tag)
                nc.vector.tensor_copy(t[:], src[:])
                return t

            u01b = tb("u01b", u01)
            u10b = tb("u10b", u10)
            u11b = tb("u11b", u11)

            idxf = t9("idxf")
            nc.vector.scalar_tensor_tensor(idxf[:], qlx[:], float(HP), qly[:],
                                           ALU.mult, ALU.add)
            idx16 = ppool.tile([128, CHUNKS, N], DT.int16, tag="idx16")
            nc.vector.tensor_copy(idx16[:], idxf[:])
            idx16b = ppool.tile([128, N, CHUNKS], DT.int16, tag="idx16b")
            nc.vector.tensor_copy(idx16b[:], idx16.transpose([0, 2, 1]))

            # ---- wrapped-index relayout (idx i at partition i%16, free i//16)
            wr = wpool.tile([128, N * 128], DT.int16, tag="wr")
            wr4 = wr.rearrange("p (n c e) -> p n c e", n=N, c=CHUNKS, e=8)
            for pg in range(8):
                nc.sync.dma_start(
                    wr4[0:16, :, :, pg],
                    idx16b[16 * pg:16 * pg + 16])
            for g in range(1, 8):
                nc.sync.dma_start(wr[16 * g:16 * g + 16, :], wr[0:16, :])

            # ---- main loop ----------------------------------------------------
            pso = acc_pool.tile([128, 2 * 1024], DT.float32, tag="acc", name="pso")
            for sh in range(2):
                for n in range(N):
                    gb = gpool.tile([128, 8, GE], DT.bfloat16, tag="gb", name="gb")
                    nc.gpsimd.dma_gather(
                        gb[:], rrows.ap(),
                        wr[:, n * 128 + sh * 64: n * 128 + sh * 64 + 64],
                        num_idxs=1024, num_idxs_reg=1024, elem_size=GE,
                        queue_num=0)
                    # corner combine: ACT does the first scale, DVE fused
                    # scalar_tensor_tensor (mul+add) chains the rest
                    xos = []
                    for ch in range(8):
                        cidx = sh * 8 + ch
                        xo0 = xopool.tile([128, C], DT.bfloat16, tag="xo0",
                                          name="xo0", bufs=4)
                        nc.scalar.mul(xo0[:], gb[:, ch, 0:C],
                                      u00[:, cidx, n:n + 1])
                        xo1 = xopool.tile([128, C], DT.bfloat16, tag="xo1",
                                          name="xo1", bufs=4)
                        nc.vector.scalar_tensor_tensor(
                            xo1[:], gb[:, ch, C:2 * C], u01b[:, cidx, n:n + 1],
                            xo0[:], ALU.mult, ALU.add)
                        xo2 = xopool.tile([128, C], DT.bfloat16, tag="xo2",
                                          name="xo2", bufs=4)
                        nc.vector.scalar_tensor_tensor(
                            xo2[:], gb[:, ch, 2 * C:3 * C], u10b[:, cidx, n:n + 1],
                            xo1[:], ALU.mult, ALU.add)
                        xo = xopool.tile([128, C], DT.bfloat16, tag="xo",
                                         name="xo", bufs=10)
                        nc.vector.scalar_tensor_tensor(
                            xo[:], gb[:, ch, 3 * C:4 * C], u11b[:, cidx, n:n + 1],
                            xo2[:], ALU.mult, ALU.add)
                        xos.append(xo)
                    for g in range(2):
                        pt = tp_pool.tile([128, 1024], DT.bfloat16, tag="tp",
                                          name="pt")
                        for ch in range(8):
                            nc.tensor.transpose(pt[:, ch * 128:(ch + 1) * 128],
                                                xos[ch][:, g * 128:(g + 1) * 128],
                                                eyeb[:])
                        xoT = xotpool.tile([128, 1024], DT.bfloat16, tag="xoT",
                                           name="xoT")
                        nc.scalar.copy(xoT[:], pt[:])
                        for og in range(2):
                            for nb in range(2):
                                nc.tensor.matmul(
                                    pso[:, og * 1024 + nb * 512:
                                        og * 1024 + (nb + 1) * 512],
                                    wcv_sb[:, n, g, og * 128:(og + 1) * 128],
                                    xoT[:, nb * 512:(nb + 1) * 512],
                                    start=(n == 0 and g == 0),
                                    stop=(n == N - 1 and g == 1))
                for og in range(2):
                    ob = opool.tile([128, 1024], DT.float32, tag="ob", name="ob")
                    nc.scalar.copy(ob[:], pso[:, og * 1024:(og + 1) * 1024])
                    nc.sync.dma_start(outd[og, :, sh * 1024:(sh + 1) * 1024], ob[:])

    nc.compile()
    return nc


def _host_prep(x, w_conv, w_p, b_p, w_m, b_m, w_ad, b_ad):
    bf16 = ml_dtypes.bfloat16
    x = np.asarray(x, dtype=np.float32)
    wsm_full = np.concatenate([np.asarray(w_p), np.asarray(w_m),
                               np.asarray(w_ad)], axis=0).astype(np.float32)
    wsm_in = np.ascontiguousarray(
        wsm_full.transpose(2, 3, 1, 0).reshape(N, 2, 128, 30))
    bias_in = np.concatenate([np.asarray(b_p), np.asarray(b_m),
                              np.asarray(b_ad)]).astype(np.float32).reshape(30, 1)
    wcv_in = np.ascontiguousarray(
        np.asarray(w_conv).astype(np.float32).transpose(2, 3, 1, 0)
        .reshape(N, 2, 128, OC)).astype(bf16)
    eyeb = np.eye(128, dtype=np.float32).astype(bf16)
    eyef = np.eye(128, dtype=np.float32)

    in_maps = []
    for k in range(NCORES):
        b, half = k // 2, k % 2
        i0 = ROWS * half
        xp = np.pad(x[b], ((0, 0), (1, 1), (1, 1)))
        slab = np.ascontiguousarray(xp[:, i0:i0 + 34, :]).reshape(2, 128, 34, HP)
        a = np.pad(x[b], ((0, 0), (1, 2), (1, 2))).astype(bf16)
        t = a.transpose(1, 2, 0)                       # (67, 67, 256)
        r4 = np.empty((HP, HP, 4, C), dtype=bf16)
        r4[:, :, 0] = t[0:HP, 0:HP]
        r4[:, :, 1] = t[0:HP, 1:HP + 1]
        r4[:, :, 2] = t[1:HP + 1, 0:HP]
        r4[:, :, 3] = t[1:HP + 1, 1:HP + 1]
        rr = r4.reshape(GROWS, GE)
        sidx = np.arange(S)
        p0x = (1.0 + i0 + sidx // W).astype(np.float32)
        p0y = (1.0 + sidx % W).astype(np.float32)
        # layout [partition, chunk, n]: s = chunk*128 + p
        p0x_t = np.ascontiguousarray(
            np.broadcast_to(p0x.reshape(CHUNKS, 128).T[:, :, None],
                            (128, CHUNKS, N)))
        p0y_t = np.ascontiguousarray(
            np.broadcast_to(p0y.reshape(CHUNKS, 128).T[:, :, None],
                            (128, CHUNKS, N)))
        in_maps.append({
            "slab": slab.astype(np.float32),
            "rrows": rr,
            "wsm": wsm_in,
            "biasd": bias_in,
            "p0xd": p0x_t,
            "p0yd": p0y_t,
            "wcv": wcv_in,
            "eyebd": eyeb,
            "eyefd": eyef,
        })
    return in_maps


def _assemble(results):
    out = np.empty((B, OC, H, W), dtype=np.float32)
    for k in range(NCORES):
        b, half = k // 2, k % 2
        i0 = ROWS * half
        o = np.asarray(results[k]["out"], dtype=np.float32)   # (2, 128, S)
        out[b, :, i0:i0 + ROWS, :] = o.reshape(OC, ROWS, W)
    return out


def run_kernel(inputs, trace=False, **trace_kwargs):
    if "nc" not in _CACHED:
        _CACHED["nc"] = _build_program()
    nc = _CACHED["nc"]
    in_maps = _host_prep(**inputs)
    res = run_bass_kernel_spmd(nc, in_maps, list(range(NCORES)), trace=trace,
                               **trace_kwargs)
    return _assemble(res.results), res


def kernel(**inputs) -> np.ndarray:
    out, _ = run_kernel(inputs)
    return out

